# revision 5
# baseline (speedup 1.0000x reference)
"""Dense MLP y = x @ W.T + b on 8 TRN2 NeuronCores, data-parallel over batch.

Full inputs: x [8192, 1024] f32, W [1024, 1024] f32, b [1024] f32.
Each core computes a [1024, 1024] slice of the output.

Per-core kernel computes the transposed output
    outT[n, m] = sum_k WT[k, n] * xT[k, m] + b[n]
so the bias lands on the partition dim (n) and fuses into the PSUM
eviction as a DVE tensor_scalar add. Host pre-transposes x-shards and W
to K-major (contraction on partitions) and un-transposes the gathered
outputs; only device time counts.

Raw Bass (no TileContext: its exit drain trips "Too many sync wait
commands" in this compiler build). Engine streams:
  sync:   33 load DMAs (bias, c0 batch, wT c1, xT c1), then 16 output
          stores gated on evictions.
  tensor: 16 PSUM groups (8 banks round-robin) x 8 fp32r matmuls
          accumulating over K.
  vector: 16 PSUM->SBUF evictions with fused bias add.
All matmul operands are float32r end to end (DRAM + SBUF) - the BIR
verifier requires fp32r matmul inputs to be produced as fp32r, and
fp32r streams 4x faster than fp32 through the PE at moving-dim 512.
"""

import numpy as np

import concourse.bass as bass
import concourse.mybir as mybir
from concourse.bass_utils import run_bass_kernel_spmd

B, IN_F, OUT_F = 8192, 1024, 1024
N_CORES = 8
M = B // N_CORES  # batch rows per core
P = 128           # partitions
MB = 512          # moving-dim block (one PSUM bank of fp32)
KT = IN_F // P    # k tiles (8)
NT = OUT_F // P   # n tiles (8)
CB = 512          # column-block width (2KB DMA lines per partition)
NGROUPS = (M // MB) * NT  # 16 psum groups, order g = mb*NT + nt

F32 = mybir.dt.float32
F32R = mybir.dt.float32r


def build_program() -> bass.Bass:
    nc = bass.Bass()
    xT = nc.declare_dram_parameter("xT", [IN_F, M], F32R, isOutput=False)
    wT = nc.declare_dram_parameter("wT", [IN_F, OUT_F], F32R, isOutput=False)
    bias = nc.declare_dram_parameter("bias", [P, NT], F32, isOutput=False)
    outT = nc.declare_dram_parameter("outT", [OUT_F, M], F32, isOutput=True)

    import contextlib

    with contextlib.ExitStack() as ctx:
        wt_sb = [
            [ctx.enter_context(nc.sbuf_tensor(f"wt{k}_{c}", [P, CB], F32R))
             for c in range(2)]
            for k in range(KT)
        ]
        xt_sb = [
            [ctx.enter_context(nc.sbuf_tensor(f"xt{k}_{c}", [P, CB], F32R))
             for c in range(2)]
            for k in range(KT)
        ]
        ot_sb = [
            ctx.enter_context(nc.sbuf_tensor(f"ot{j}", [P, MB], F32))
            for j in range(4)
        ]
        bias_sb = ctx.enter_context(nc.sbuf_tensor("bias_sb", [P, NT], F32))
        ps = [
            ctx.enter_context(nc.psum_tensor(f"ps{b}", [P, MB], F32))
            for b in range(8)
        ]
        ld_b = ctx.enter_context(nc.semaphore("ld_b"))
        ld_c0 = ctx.enter_context(nc.semaphore("ld_c0"))
        ld_w1 = ctx.enter_context(nc.semaphore("ld_w1"))
        ld_x1 = ctx.enter_context(nc.semaphore("ld_x1"))
        mm = ctx.enter_context(nc.semaphore("mm"))
        ev = ctx.enter_context(nc.semaphore("ev"))
        # One store-completion sem per ot slot: DMA completions are
        # unordered across transfers, so a shared counter can't prove a
        # *specific* store finished — a per-slot counter can (slot-j
        # stores are the only incrementers, and all issued must complete
        # for the sum to reach the target).
        st_sems = [
            ctx.enter_context(nc.semaphore(f"st{j}")) for j in range(4)
        ]

        with nc.Block() as block:

            @block.sync
            def _(sync):
                sync.dma_start(out=bias_sb[:], in_=bias[:]).then_inc(ld_b, 16)
                for k in range(KT):
                    sync.dma_start(
                        out=wt_sb[k][0][:],
                        in_=wT[k * P:(k + 1) * P, 0:CB],
                    ).then_inc(ld_c0, 16)
                    sync.dma_start(
                        out=xt_sb[k][0][:],
                        in_=xT[k * P:(k + 1) * P, 0:CB],
                    ).then_inc(ld_c0, 16)
                for k in range(KT):
                    sync.dma_start(
                        out=wt_sb[k][1][:],
                        in_=wT[k * P:(k + 1) * P, CB:2 * CB],
                    ).then_inc(ld_w1, 16)
                for k in range(KT):
                    sync.dma_start(
                        out=xt_sb[k][1][:],
                        in_=xT[k * P:(k + 1) * P, CB:2 * CB],
                    ).then_inc(ld_x1, 16)
                for g in range(NGROUPS):
                    mb, nt = divmod(g, NT)
                    sync.wait_ge(ev, g + 1)
                    sync.dma_start(
                        out=outT[nt * P:(nt + 1) * P, mb * MB:(mb + 1) * MB],
                        in_=ot_sb[g % 4][:],
                    ).then_inc(st_sems[g % 4], 16)
                for j in range(4):
                    sync.wait_ge(st_sems[j], (NGROUPS // 4) * 16)

            @block.tensor
            def _(tensor):
                tensor.wait_ge(ld_c0, 256)
                for g in range(NGROUPS):
                    mb, nt = divmod(g, NT)
                    if g == 4:
                        tensor.wait_ge(ld_w1, 128)
                    if g == 8:
                        tensor.wait_ge(ld_x1, 128)
                    if g >= 8:
                        # psum bank g%8 reused from group g-8
                        tensor.wait_ge(ev, g - 7)
                    c, ni = divmod(nt, CB // P)
                    inst = None
                    for k in range(KT):
                        inst = tensor.matmul(
                            ps[g % 8][:, :],
                            wt_sb[k][c][:, ni * P:(ni + 1) * P],
                            xt_sb[k][mb][:, :],
                            start=(k == 0),
                            stop=(k == KT - 1),
                        )
                    inst.then_inc(mm, 1)

            @block.vector
            def _(vector):
                vector.wait_ge(ld_b, 16)
                for g in range(NGROUPS):
                    mb, nt = divmod(g, NT)
                    vector.wait_ge(mm, g + 1)
                    if g >= 4:
                        # ot slot g%4 reused: all issued slot stores
                        # (groups g%4, g%4+4, ..., g-4) must be done
                        vector.wait_ge(st_sems[g % 4], (g // 4) * 16)
                    vector.tensor_scalar_add(
                        ot_sb[g % 4][:],
                        ps[g % 8][:, :],
                        bias_sb[:, nt:nt + 1],
                    ).then_inc(ev, 1)

    return nc


_PROGRAM = None


def _get_program() -> bass.Bass:
    global _PROGRAM
    if _PROGRAM is None:
        _PROGRAM = build_program()
    return _PROGRAM


def make_in_maps(x: np.ndarray, W: np.ndarray, b: np.ndarray) -> list[dict]:
    WT = np.ascontiguousarray(W.T.astype(np.float32, copy=False))
    bias = np.ascontiguousarray(
        b.astype(np.float32, copy=False).reshape(NT, P).T
    )
    in_maps = []
    for c in range(N_CORES):
        xT = np.ascontiguousarray(
            x[c * M:(c + 1) * M, :].T.astype(np.float32, copy=False)
        )
        in_maps.append({"xT": xT, "wT": WT, "bias": bias})
    return in_maps


def assemble_output(results: list[dict]) -> np.ndarray:
    out = np.empty((B, OUT_F), dtype=np.float32)
    for c in range(N_CORES):
        out[c * M:(c + 1) * M, :] = results[c]["outT"].T
    return out


def kernel(x: np.ndarray, W: np.ndarray, b: np.ndarray) -> np.ndarray:
    nc = _get_program()
    in_maps = make_in_maps(np.asarray(x), np.asarray(W), np.asarray(b))
    res = run_bass_kernel_spmd(nc, in_maps, list(range(N_CORES)))
    return assemble_output(res.results)
